# revision 8
# baseline (speedup 1.0000x reference)
"""CompressedLinear Trainium2 kernel.

Computes out[b,s,o] = x[b,s,i] @ (int8_weight[o,i] * scale).T + bias[o]
with x: [4,2048,4096] f32, weight_int8: [11008,4096] int32 (int8 values),
scale: scalar f32, bias: [11008] f32.

Sharding: column-parallel over 8 NeuronCores — each core owns 1376
out-features (weight + bias slice), x is replicated, outputs concat on
the last dim.

Per-core device kernel (Bass/Tile):
  - weight slice is uploaded in [in, out] layout (int32); the device
    dequantizes shard-locally: SWDGE cast-DMA int32 -> bf16 (exact for
    int8-range values) into a resident SBUF tile [4096 x 1376].
  - x is uploaded in [in, s] layout (f32); streamed as SWDGE cast-DMA
    f32 -> bf16 tiles.
  - TensorE: psum[s=128, o<=512] += xT_tile[k,s].T-free @ wT_tile[k,o]
    accumulated over 32 k-tiles of 128.
  - epilogue (DVE): out = psum * scale + bias in one scalar_tensor_tensor,
    then HWDGE store to DRAM in natural [s, o] layout.
"""

import numpy as np

import concourse.bacc as bacc
import concourse.mybir as mybir
import concourse.tile as tile
from concourse.bass_utils import run_bass_kernel_spmd

# Problem shape (hardcoded per contract)
B, S, IN_F, OUT_F = 4, 2048, 4096, 11008
NCORES = 8
OUT_PER = OUT_F // NCORES  # 1376
S_TOT = B * S  # 8192

# Tiling
KTILE = 128  # contraction per matmul
S_CHUNK = 512  # s-columns per x-load group
S_SUB = 128  # out-rows per psum block
KGRP = 4  # k-tiles per x DMA (1 MiB f32 reads)
NMAX = 512  # max moving free dim / psum bank

# set by test harness to capture profiles; harness calls kernel() untouched
TRACE = False
LAST_RESULT = None

_cache = {}


def _n_chunks(out_per):
    chunks = []
    off = 0
    while off < out_per:
        sz = min(NMAX, out_per - off)
        chunks.append((off, sz))
        off += sz
    return chunks


def build_nc(s_tot=S_TOT, in_f=IN_F, out_per=OUT_PER, s_chunk=S_CHUNK, kgrp=KGRP):
    f32 = mybir.dt.float32
    bf16 = mybir.dt.bfloat16
    i8 = mybir.dt.int8

    KT = in_f // KTILE  # k-tiles
    NKG = KT // kgrp  # x-load groups per s-chunk
    chunks = _n_chunks(out_per)

    nc = bacc.Bacc("TRN2", target_bir_lowering=False, debug=False, num_devices=NCORES)

    xt = nc.dram_tensor("xt", [in_f, s_tot], f32, kind="ExternalInput").ap()
    wt = nc.dram_tensor("wt", [in_f, out_per], i8, kind="ExternalInput").ap()
    bias = nc.dram_tensor("bias", [1, out_per], f32, kind="ExternalInput").ap()
    scale = nc.dram_tensor("scale", [1, 1], f32, kind="ExternalInput").ap()
    out = nc.dram_tensor("out", [s_tot, out_per], f32, kind="ExternalOutput").ap()

    # s-chunk schedule: narrow warmup chunks so the first psum blocks aren't
    # gated on the full 8 MB x-chunk + 5.6 MB weight load.
    warm = min(s_chunk // 2, 256)
    if s_tot > 2 * warm and (s_tot - 2 * warm) % s_chunk == 0:
        chunk_sched = [warm, warm] + [s_chunk] * ((s_tot - 2 * warm) // s_chunk)
    else:
        chunk_sched = [s_chunk] * (s_tot // s_chunk)

    with tile.TileContext(nc) as tc:
        with (
            tc.tile_pool(name="wt", bufs=1) as wt_pool,
            tc.tile_pool(name="xbf", bufs=2 * NKG) as xbf_pool,
            tc.tile_pool(name="psum", bufs=2, space="PSUM") as psum_pool,
            tc.tile_pool(name="osb", bufs=3) as osb_pool,
            tc.tile_pool(name="consts", bufs=1) as const_pool,
        ):
            # Startup: interleave weight dequant (int8 -> bf16 cast DMA, exact
            # for int8-range values) with the first s-chunk's x loads, x tile
            # first — the tensor engine needs (xg0, wtg0) for its first MM.
            wts = [
                wt_pool.tile([128, kgrp, out_per], bf16, tag=f"wt{g}", name=f"wt{g}")
                for g in range(NKG)
            ]
            sc0 = chunk_sched[0]
            xg0 = []
            for g in range(NKG):
                t = xbf_pool.tile([128, kgrp, sc0], bf16, tag="xbf", name=f"x0_{g}")
                src = xt[g * kgrp * 128 : (g + 1) * kgrp * 128, 0:sc0].rearrange(
                    "(g p) s -> p g s", p=128
                )
                nc.gpsimd.dma_start(out=t[:], in_=src)
                xg0.append(t)
                wsrc = wt[g * kgrp * 128 : (g + 1) * kgrp * 128, :].rearrange(
                    "(g p) o -> p g o", p=128
                )
                nc.gpsimd.dma_start(out=wts[g][:], in_=wsrc)

            scale_sb = const_pool.tile([128, 1], f32, tag="scale", name="scale_sb")
            nc.sync.dma_start(out=scale_sb[:], in_=scale.partition_broadcast(128))
            bias_sb = const_pool.tile([128, out_per], f32, tag="bias", name="bias_sb")
            nc.sync.dma_start(out=bias_sb[:], in_=bias.partition_broadcast(128))

            s0 = 0
            for ci, sc in enumerate(chunk_sched):
                if ci == 0:
                    xg = xg0
                else:
                    # x chunk load: cast f32 -> bf16 in DMA, [128, kgrp, sc]
                    xg = []
                    for g in range(NKG):
                        t = xbf_pool.tile(
                            [128, kgrp, sc], bf16, tag="xbf", name=f"x{ci}_{g}"
                        )
                        src = xt[
                            g * kgrp * 128 : (g + 1) * kgrp * 128, s0 : s0 + sc
                        ].rearrange("(g p) s -> p g s", p=128)
                        nc.gpsimd.dma_start(out=t[:], in_=src)
                        xg.append(t)

                for sub in range(sc // S_SUB):
                    psums = [
                        psum_pool.tile(
                            [128, NMAX],
                            f32,
                            tag=f"ps{j}",
                            name=f"ps{ci}_{sub}_{j}",
                            bufs=3 if j < 2 else 2,
                        )
                        for j in range(len(chunks))
                    ]
                    for k in range(KT):
                        lhsT = xg[k // kgrp][:, k % kgrp, sub * 128 : (sub + 1) * 128]
                        for j, (off, sz) in enumerate(chunks):
                            nc.tensor.matmul(
                                psums[j][:, :sz],
                                lhsT,
                                wts[k // kgrp][:, k % kgrp, off : off + sz],
                                start=(k == 0),
                                stop=(k == KT - 1),
                            )
                    osb = osb_pool.tile(
                        [128, out_per], f32, tag="osb", name=f"o{ci}_{sub}"
                    )
                    r0 = s0 + sub * S_SUB
                    for j, (off, sz) in enumerate(chunks):
                        nc.vector.scalar_tensor_tensor(
                            osb[:, off : off + sz],
                            psums[j][:, :sz],
                            scale_sb[:, 0:1],
                            bias_sb[:, off : off + sz],
                            mybir.AluOpType.mult,
                            mybir.AluOpType.add,
                        )
                        nc.sync.dma_start(
                            out=out[r0 : r0 + S_SUB, off : off + sz],
                            in_=osb[:, off : off + sz],
                        )
                s0 += sc

    nc.compile()
    return nc


def _get_nc():
    key = "full"
    if key not in _cache:
        _cache[key] = build_nc()
    return _cache[key]


def kernel(x, weight_int8, scale, bias):
    global LAST_RESULT
    x = np.asarray(x, dtype=np.float32)
    w = np.asarray(weight_int8)
    scale_f = np.float32(np.asarray(scale).reshape(()))
    bias = np.asarray(bias, dtype=np.float32)

    # host-side layout prep (sharding): contraction dim to the front; the
    # int8-valued weight is shipped in its compressed (int8) form
    xt = np.ascontiguousarray(x.reshape(S_TOT, IN_F).T)  # [in, s]
    wt_full = np.ascontiguousarray(w.T.astype(np.int8))  # [in, out]
    scale_rep = np.full((1, 1), scale_f, dtype=np.float32)

    nc = _get_nc()
    in_maps = []
    for c in range(NCORES):
        o0, o1 = c * OUT_PER, (c + 1) * OUT_PER
        in_maps.append(
            {
                "xt": xt,
                "wt": np.ascontiguousarray(wt_full[:, o0:o1]),
                "bias": np.ascontiguousarray(bias[o0:o1][None, :]),
                "scale": scale_rep,
            }
        )

    res = run_bass_kernel_spmd(
        nc, in_maps, core_ids=list(range(NCORES)), trace=TRACE
    )
    LAST_RESULT = res
    out = np.concatenate([res.results[c]["out"] for c in range(NCORES)], axis=1)
    return out.reshape(B, S, OUT_F)


# revision 12
# speedup vs baseline: 1.0047x; 1.0047x over previous
"""CompressedLinear Trainium2 kernel.

Computes out[b,s,o] = x[b,s,i] @ (int8_weight[o,i] * scale).T + bias[o]
with x: [4,2048,4096] f32, weight_int8: [11008,4096] int32 (int8 values),
scale: scalar f32, bias: [11008] f32.

Sharding: column-parallel over 8 NeuronCores — each core owns 1376
out-features (weight + bias slice), x is replicated, outputs concat on
the last dim.

Per-core device kernel (Bass/Tile):
  - weight slice is uploaded in [in, out] layout (int32); the device
    dequantizes shard-locally: SWDGE cast-DMA int32 -> bf16 (exact for
    int8-range values) into a resident SBUF tile [4096 x 1376].
  - x is uploaded in [in, s] layout (f32); streamed as SWDGE cast-DMA
    f32 -> bf16 tiles.
  - TensorE: psum[s=128, o<=512] += xT_tile[k,s].T-free @ wT_tile[k,o]
    accumulated over 32 k-tiles of 128.
  - epilogue (DVE): out = psum * scale + bias in one scalar_tensor_tensor,
    then HWDGE store to DRAM in natural [s, o] layout.
"""

import numpy as np

import concourse.bacc as bacc
import concourse.mybir as mybir
import concourse.tile as tile
from concourse.bass_utils import run_bass_kernel_spmd

# Problem shape (hardcoded per contract)
B, S, IN_F, OUT_F = 4, 2048, 4096, 11008
NCORES = 8
OUT_PER = OUT_F // NCORES  # 1376
S_TOT = B * S  # 8192

# Tiling
KTILE = 128  # contraction per matmul
S_CHUNK = 512  # s-columns per x-load group
S_SUB = 128  # out-rows per psum block
KGRP = 4  # k-tiles per x DMA (1 MiB f32 reads)
NMAX = 512  # max moving free dim / psum bank

# set by test harness to capture profiles; harness calls kernel() untouched
TRACE = False
LAST_RESULT = None

_cache = {}


def _n_chunks(out_per):
    chunks = []
    off = 0
    while off < out_per:
        sz = min(NMAX, out_per - off)
        chunks.append((off, sz))
        off += sz
    return chunks


def build_nc(s_tot=S_TOT, in_f=IN_F, out_per=OUT_PER, s_chunk=S_CHUNK, kgrp=KGRP):
    f32 = mybir.dt.float32
    bf16 = mybir.dt.bfloat16
    i8 = mybir.dt.int8

    KT = in_f // KTILE  # k-tiles
    NKG = KT // kgrp  # x-load groups per s-chunk
    chunks = _n_chunks(out_per)

    nc = bacc.Bacc("TRN2", target_bir_lowering=False, debug=False, num_devices=NCORES)

    xt = nc.dram_tensor("xt", [in_f, s_tot], f32, kind="ExternalInput").ap()
    wt = nc.dram_tensor("wt", [in_f, out_per], i8, kind="ExternalInput").ap()
    bias = nc.dram_tensor("bias", [1, out_per], f32, kind="ExternalInput").ap()
    scale = nc.dram_tensor("scale", [1, 1], f32, kind="ExternalInput").ap()
    out = nc.dram_tensor("out", [s_tot, out_per], f32, kind="ExternalOutput").ap()

    # s-chunk schedule: narrow warmup chunks so the first psum blocks aren't
    # gated on the full 8 MB x-chunk + 5.6 MB weight load.
    warm = min(s_chunk // 2, 256)
    if s_tot > 2 * warm and (s_tot - 2 * warm) % s_chunk == 0:
        chunk_sched = [warm, warm] + [s_chunk] * ((s_tot - 2 * warm) // s_chunk)
    else:
        chunk_sched = [s_chunk] * (s_tot // s_chunk)

    with tile.TileContext(nc) as tc:
        with (
            tc.tile_pool(name="wt", bufs=1) as wt_pool,
            tc.tile_pool(name="xbf", bufs=2 * NKG + 1) as xbf_pool,
            tc.tile_pool(name="psum", bufs=2, space="PSUM") as psum_pool,
            tc.tile_pool(name="osb", bufs=3) as osb_pool,
            tc.tile_pool(name="consts", bufs=1) as const_pool,
        ):
            # Startup: interleave weight dequant (int8 -> bf16 cast DMA, exact
            # for int8-range values) with the first s-chunk's x loads, x tile
            # first — the tensor engine needs (xg0, wtg0) for its first MM.
            # The very first (x, w) pair covers a single k-tile so the first
            # matmul's dependencies are a few hundred KB, not MBs.
            groups0 = [(0, 1), (1, kgrp - 1)] + [
                (g * kgrp, kgrp) for g in range(1, NKG)
            ]
            sc0 = chunk_sched[0]
            wtk = {}  # k -> (tile, idx within tile)
            xg0 = {}
            for gi, (k0, kn) in enumerate(groups0):
                t = xbf_pool.tile([128, kn, sc0], bf16, tag="xbf", name=f"x0_{gi}")
                src = xt[k0 * 128 : (k0 + kn) * 128, 0:sc0].rearrange(
                    "(g p) s -> p g s", p=128
                )
                nc.gpsimd.dma_start(out=t[:], in_=src)
                for i in range(kn):
                    xg0[k0 + i] = (t, i)
                wtile = wt_pool.tile(
                    [128, kn, out_per], bf16, tag=f"wt{gi}", name=f"wt{gi}"
                )
                wsrc = wt[k0 * 128 : (k0 + kn) * 128, :].rearrange(
                    "(g p) o -> p g o", p=128
                )
                nc.gpsimd.dma_start(out=wtile[:], in_=wsrc)
                for i in range(kn):
                    wtk[k0 + i] = (wtile, i)

            scale_sb = const_pool.tile([128, 1], f32, tag="scale", name="scale_sb")
            nc.sync.dma_start(out=scale_sb[:], in_=scale.partition_broadcast(128))
            bias_sb = const_pool.tile([128, out_per], f32, tag="bias", name="bias_sb")
            nc.sync.dma_start(out=bias_sb[:], in_=bias.partition_broadcast(128))

            s0 = 0
            for ci, sc in enumerate(chunk_sched):
                if ci == 0:
                    xg = xg0
                else:
                    # x chunk load: cast f32 -> bf16 in DMA, [128, kgrp, sc]
                    xg = {}
                    for g in range(NKG):
                        t = xbf_pool.tile(
                            [128, kgrp, sc], bf16, tag="xbf", name=f"x{ci}_{g}"
                        )
                        src = xt[
                            g * kgrp * 128 : (g + 1) * kgrp * 128, s0 : s0 + sc
                        ].rearrange("(g p) s -> p g s", p=128)
                        nc.gpsimd.dma_start(out=t[:], in_=src)
                        for i in range(kgrp):
                            xg[g * kgrp + i] = (t, i)

                for sub in range(sc // S_SUB):
                    psums = [
                        psum_pool.tile(
                            [128, NMAX], f32, tag=f"ps{j}", name=f"ps{ci}_{sub}_{j}"
                        )
                        for j in range(len(chunks))
                    ]
                    for k in range(KT):
                        xt_t, xi = xg[k]
                        w_t, wi = wtk[k]
                        lhsT = xt_t[:, xi, sub * 128 : (sub + 1) * 128]
                        for j, (off, sz) in enumerate(chunks):
                            nc.tensor.matmul(
                                psums[j][:, :sz],
                                lhsT,
                                w_t[:, wi, off : off + sz],
                                start=(k == 0),
                                stop=(k == KT - 1),
                            )
                    osb = osb_pool.tile(
                        [128, out_per], f32, tag="osb", name=f"o{ci}_{sub}"
                    )
                    r0 = s0 + sub * S_SUB
                    for j, (off, sz) in enumerate(chunks):
                        nc.vector.scalar_tensor_tensor(
                            osb[:, off : off + sz],
                            psums[j][:, :sz],
                            scale_sb[:, 0:1],
                            bias_sb[:, off : off + sz],
                            mybir.AluOpType.mult,
                            mybir.AluOpType.add,
                        )
                        nc.sync.dma_start(
                            out=out[r0 : r0 + S_SUB, off : off + sz],
                            in_=osb[:, off : off + sz],
                        )
                s0 += sc

    nc.compile()
    return nc


def _get_nc():
    key = "full"
    if key not in _cache:
        _cache[key] = build_nc()
    return _cache[key]


def kernel(x, weight_int8, scale, bias):
    global LAST_RESULT
    x = np.asarray(x, dtype=np.float32)
    w = np.asarray(weight_int8)
    scale_f = np.float32(np.asarray(scale).reshape(()))
    bias = np.asarray(bias, dtype=np.float32)

    # host-side layout prep (sharding): contraction dim to the front; the
    # int8-valued weight is shipped in its compressed (int8) form
    xt = np.ascontiguousarray(x.reshape(S_TOT, IN_F).T)  # [in, s]
    wt_full = np.ascontiguousarray(w.T.astype(np.int8))  # [in, out]
    scale_rep = np.full((1, 1), scale_f, dtype=np.float32)

    nc = _get_nc()
    in_maps = []
    for c in range(NCORES):
        o0, o1 = c * OUT_PER, (c + 1) * OUT_PER
        in_maps.append(
            {
                "xt": xt,
                "wt": np.ascontiguousarray(wt_full[:, o0:o1]),
                "bias": np.ascontiguousarray(bias[o0:o1][None, :]),
                "scale": scale_rep,
            }
        )

    res = run_bass_kernel_spmd(
        nc, in_maps, core_ids=list(range(NCORES)), trace=TRACE
    )
    LAST_RESULT = res
    out = np.concatenate([res.results[c]["out"] for c in range(NCORES)], axis=1)
    return out.reshape(B, S, OUT_F)


# revision 13
# speedup vs baseline: 1.0051x; 1.0004x over previous
"""CompressedLinear Trainium2 kernel.

Computes out[b,s,o] = x[b,s,i] @ (int8_weight[o,i] * scale).T + bias[o]
with x: [4,2048,4096] f32, weight_int8: [11008,4096] int32 (int8 values),
scale: scalar f32, bias: [11008] f32.

Sharding: column-parallel over 8 NeuronCores — each core owns 1376
out-features (weight + bias slice), x is replicated, outputs concat on
the last dim.

Per-core device kernel (Bass/Tile):
  - weight slice is uploaded in [in, out] layout (int32); the device
    dequantizes shard-locally: SWDGE cast-DMA int32 -> bf16 (exact for
    int8-range values) into a resident SBUF tile [4096 x 1376].
  - x is uploaded in [in, s] layout (f32); streamed as SWDGE cast-DMA
    f32 -> bf16 tiles.
  - TensorE: psum[s=128, o<=512] += xT_tile[k,s].T-free @ wT_tile[k,o]
    accumulated over 32 k-tiles of 128.
  - epilogue (DVE): out = psum * scale + bias in one scalar_tensor_tensor,
    then HWDGE store to DRAM in natural [s, o] layout.
"""

import numpy as np

import concourse.bacc as bacc
import concourse.mybir as mybir
import concourse.tile as tile
from concourse.bass_utils import run_bass_kernel_spmd

# Problem shape (hardcoded per contract)
B, S, IN_F, OUT_F = 4, 2048, 4096, 11008
NCORES = 8
OUT_PER = OUT_F // NCORES  # 1376
S_TOT = B * S  # 8192

# Tiling
KTILE = 128  # contraction per matmul
S_CHUNK = 512  # s-columns per x-load group
S_SUB = 128  # out-rows per psum block
KGRP = 4  # k-tiles per x DMA (1 MiB f32 reads)
NMAX = 512  # max moving free dim / psum bank

# set by test harness to capture profiles; harness calls kernel() untouched
TRACE = False
LAST_RESULT = None

_cache = {}


def _n_chunks(out_per):
    chunks = []
    off = 0
    while off < out_per:
        sz = min(NMAX, out_per - off)
        chunks.append((off, sz))
        off += sz
    return chunks


def build_nc(s_tot=S_TOT, in_f=IN_F, out_per=OUT_PER, s_chunk=S_CHUNK, kgrp=KGRP):
    f32 = mybir.dt.float32
    bf16 = mybir.dt.bfloat16
    i8 = mybir.dt.int8

    KT = in_f // KTILE  # k-tiles
    NKG = KT // kgrp  # x-load groups per s-chunk
    chunks = _n_chunks(out_per)

    nc = bacc.Bacc("TRN2", target_bir_lowering=False, debug=False, num_devices=NCORES)

    xt = nc.dram_tensor("xt", [in_f, s_tot], f32, kind="ExternalInput").ap()
    wt = nc.dram_tensor("wt", [in_f, out_per], i8, kind="ExternalInput").ap()
    bias = nc.dram_tensor("bias", [1, out_per], f32, kind="ExternalInput").ap()
    scale = nc.dram_tensor("scale", [1, 1], f32, kind="ExternalInput").ap()
    out = nc.dram_tensor("out", [s_tot, out_per], f32, kind="ExternalOutput").ap()

    # s-chunk schedule: narrow warmup chunks so the first psum blocks aren't
    # gated on the full 8 MB x-chunk + 5.6 MB weight load.
    warm = min(s_chunk // 2, 256)
    if s_tot > 2 * warm and (s_tot - 2 * warm) % s_chunk == 0:
        chunk_sched = [warm, warm] + [s_chunk] * ((s_tot - 2 * warm) // s_chunk)
    else:
        chunk_sched = [s_chunk] * (s_tot // s_chunk)

    with tile.TileContext(nc) as tc:
        with (
            tc.tile_pool(name="wt", bufs=1) as wt_pool,
            tc.tile_pool(name="xbf", bufs=2 * NKG + 1) as xbf_pool,
            tc.tile_pool(name="psum", bufs=2, space="PSUM") as psum_pool,
            tc.tile_pool(name="osb", bufs=3) as osb_pool,
            tc.tile_pool(name="consts", bufs=1) as const_pool,
        ):
            # HAM warmup: dummy matmuls on zeroed SBUF while the first loads
            # are in flight, so the PE clock-gate (4/8 cold -> 8/8 warm after
            # ~3.4us of activity) opens before real matmuls start.
            zeros = const_pool.tile([128, NMAX], bf16, tag="zeros", name="zeros")
            nc.vector.memset(zeros[:], 0)
            psw = psum_pool.tile([128, NMAX], f32, tag="warm", name="warm", bufs=1)
            N_WARM = 16
            for i in range(N_WARM):
                nc.tensor.matmul(
                    psw[:, :],
                    zeros[:, 0:128],
                    zeros[:, :],
                    start=(i == 0),
                    stop=(i == N_WARM - 1),
                )

            # Startup: interleave weight dequant (int8 -> bf16 cast DMA, exact
            # for int8-range values) with the first s-chunk's x loads, x tile
            # first — the tensor engine needs (xg0, wtg0) for its first MM.
            # The very first (x, w) pair covers a single k-tile so the first
            # matmul's dependencies are a few hundred KB, not MBs.
            groups0 = [(0, 1), (1, kgrp - 1)] + [
                (g * kgrp, kgrp) for g in range(1, NKG)
            ]
            sc0 = chunk_sched[0]
            wtk = {}  # k -> (tile, idx within tile)
            xg0 = {}
            for gi, (k0, kn) in enumerate(groups0):
                t = xbf_pool.tile([128, kn, sc0], bf16, tag="xbf", name=f"x0_{gi}")
                src = xt[k0 * 128 : (k0 + kn) * 128, 0:sc0].rearrange(
                    "(g p) s -> p g s", p=128
                )
                nc.gpsimd.dma_start(out=t[:], in_=src)
                for i in range(kn):
                    xg0[k0 + i] = (t, i)
                wtile = wt_pool.tile(
                    [128, kn, out_per], bf16, tag=f"wt{gi}", name=f"wt{gi}"
                )
                wsrc = wt[k0 * 128 : (k0 + kn) * 128, :].rearrange(
                    "(g p) o -> p g o", p=128
                )
                nc.gpsimd.dma_start(out=wtile[:], in_=wsrc)
                for i in range(kn):
                    wtk[k0 + i] = (wtile, i)

            scale_sb = const_pool.tile([128, 1], f32, tag="scale", name="scale_sb")
            nc.sync.dma_start(out=scale_sb[:], in_=scale.partition_broadcast(128))
            bias_sb = const_pool.tile([128, out_per], f32, tag="bias", name="bias_sb")
            nc.sync.dma_start(out=bias_sb[:], in_=bias.partition_broadcast(128))

            s0 = 0
            for ci, sc in enumerate(chunk_sched):
                if ci == 0:
                    xg = xg0
                else:
                    # x chunk load: cast f32 -> bf16 in DMA, [128, kgrp, sc]
                    xg = {}
                    for g in range(NKG):
                        t = xbf_pool.tile(
                            [128, kgrp, sc], bf16, tag="xbf", name=f"x{ci}_{g}"
                        )
                        src = xt[
                            g * kgrp * 128 : (g + 1) * kgrp * 128, s0 : s0 + sc
                        ].rearrange("(g p) s -> p g s", p=128)
                        nc.gpsimd.dma_start(out=t[:], in_=src)
                        for i in range(kgrp):
                            xg[g * kgrp + i] = (t, i)

                for sub in range(sc // S_SUB):
                    psums = [
                        psum_pool.tile(
                            [128, NMAX], f32, tag=f"ps{j}", name=f"ps{ci}_{sub}_{j}"
                        )
                        for j in range(len(chunks))
                    ]
                    for k in range(KT):
                        xt_t, xi = xg[k]
                        w_t, wi = wtk[k]
                        lhsT = xt_t[:, xi, sub * 128 : (sub + 1) * 128]
                        for j, (off, sz) in enumerate(chunks):
                            nc.tensor.matmul(
                                psums[j][:, :sz],
                                lhsT,
                                w_t[:, wi, off : off + sz],
                                start=(k == 0),
                                stop=(k == KT - 1),
                            )
                    osb = osb_pool.tile(
                        [128, out_per], f32, tag="osb", name=f"o{ci}_{sub}"
                    )
                    r0 = s0 + sub * S_SUB
                    for j, (off, sz) in enumerate(chunks):
                        nc.vector.scalar_tensor_tensor(
                            osb[:, off : off + sz],
                            psums[j][:, :sz],
                            scale_sb[:, 0:1],
                            bias_sb[:, off : off + sz],
                            mybir.AluOpType.mult,
                            mybir.AluOpType.add,
                        )
                        nc.sync.dma_start(
                            out=out[r0 : r0 + S_SUB, off : off + sz],
                            in_=osb[:, off : off + sz],
                        )
                s0 += sc

    nc.compile()
    return nc


def _get_nc():
    key = "full"
    if key not in _cache:
        _cache[key] = build_nc()
    return _cache[key]


def kernel(x, weight_int8, scale, bias):
    global LAST_RESULT
    x = np.asarray(x, dtype=np.float32)
    w = np.asarray(weight_int8)
    scale_f = np.float32(np.asarray(scale).reshape(()))
    bias = np.asarray(bias, dtype=np.float32)

    # host-side layout prep (sharding): contraction dim to the front; the
    # int8-valued weight is shipped in its compressed (int8) form
    xt = np.ascontiguousarray(x.reshape(S_TOT, IN_F).T)  # [in, s]
    wt_full = np.ascontiguousarray(w.T.astype(np.int8))  # [in, out]
    scale_rep = np.full((1, 1), scale_f, dtype=np.float32)

    nc = _get_nc()
    in_maps = []
    for c in range(NCORES):
        o0, o1 = c * OUT_PER, (c + 1) * OUT_PER
        in_maps.append(
            {
                "xt": xt,
                "wt": np.ascontiguousarray(wt_full[:, o0:o1]),
                "bias": np.ascontiguousarray(bias[o0:o1][None, :]),
                "scale": scale_rep,
            }
        )

    res = run_bass_kernel_spmd(
        nc, in_maps, core_ids=list(range(NCORES)), trace=TRACE
    )
    LAST_RESULT = res
    out = np.concatenate([res.results[c]["out"] for c in range(NCORES)], axis=1)
    return out.reshape(B, S, OUT_F)


# revision 14
# speedup vs baseline: 1.0053x; 1.0002x over previous
"""CompressedLinear Trainium2 kernel.

Computes out[b,s,o] = x[b,s,i] @ (int8_weight[o,i] * scale).T + bias[o]
with x: [4,2048,4096] f32, weight_int8: [11008,4096] int32 (int8 values),
scale: scalar f32, bias: [11008] f32.

Sharding: column-parallel over 8 NeuronCores — each core owns 1376
out-features (weight + bias slice), x is replicated, outputs concat on
the last dim.

Per-core device kernel (Bass/Tile):
  - weight slice is uploaded in [in, out] layout (int32); the device
    dequantizes shard-locally: SWDGE cast-DMA int32 -> bf16 (exact for
    int8-range values) into a resident SBUF tile [4096 x 1376].
  - x is uploaded in [in, s] layout (f32); streamed as SWDGE cast-DMA
    f32 -> bf16 tiles.
  - TensorE: psum[s=128, o<=512] += xT_tile[k,s].T-free @ wT_tile[k,o]
    accumulated over 32 k-tiles of 128.
  - epilogue (DVE): out = psum * scale + bias in one scalar_tensor_tensor,
    then HWDGE store to DRAM in natural [s, o] layout.
"""

import numpy as np

import concourse.bacc as bacc
import concourse.mybir as mybir
import concourse.tile as tile
from concourse.bass_utils import run_bass_kernel_spmd

# Problem shape (hardcoded per contract)
B, S, IN_F, OUT_F = 4, 2048, 4096, 11008
NCORES = 8
OUT_PER = OUT_F // NCORES  # 1376
S_TOT = B * S  # 8192

# Tiling
KTILE = 128  # contraction per matmul
S_CHUNK = 512  # s-columns per x-load group
S_SUB = 128  # out-rows per psum block
KGRP = 4  # k-tiles per x DMA (1 MiB f32 reads)
NMAX = 512  # max moving free dim / psum bank

# set by test harness to capture profiles; harness calls kernel() untouched
TRACE = False
LAST_RESULT = None

_cache = {}


def _n_chunks(out_per):
    chunks = []
    off = 0
    while off < out_per:
        sz = min(NMAX, out_per - off)
        chunks.append((off, sz))
        off += sz
    return chunks


def build_nc(s_tot=S_TOT, in_f=IN_F, out_per=OUT_PER, s_chunk=S_CHUNK, kgrp=KGRP):
    f32 = mybir.dt.float32
    bf16 = mybir.dt.bfloat16
    i8 = mybir.dt.int8

    KT = in_f // KTILE  # k-tiles
    NKG = KT // kgrp  # x-load groups per s-chunk
    chunks = _n_chunks(out_per)

    nc = bacc.Bacc("TRN2", target_bir_lowering=False, debug=False, num_devices=NCORES)

    xt = nc.dram_tensor("xt", [in_f, s_tot], f32, kind="ExternalInput").ap()
    wt = nc.dram_tensor("wt", [in_f, out_per], i8, kind="ExternalInput").ap()
    bias = nc.dram_tensor("bias", [1, out_per], f32, kind="ExternalInput").ap()
    scale = nc.dram_tensor("scale", [1, 1], f32, kind="ExternalInput").ap()
    out = nc.dram_tensor("out", [s_tot, out_per], f32, kind="ExternalOutput").ap()

    # s-chunk schedule: narrow warmup chunks so the first psum blocks aren't
    # gated on the full 8 MB x-chunk + 5.6 MB weight load.
    warm = min(s_chunk // 2, 256)
    if s_tot > 2 * warm and (s_tot - 2 * warm) % s_chunk == 0:
        chunk_sched = [warm, warm] + [s_chunk] * ((s_tot - 2 * warm) // s_chunk)
    else:
        chunk_sched = [s_chunk] * (s_tot // s_chunk)

    with tile.TileContext(nc) as tc:
        with (
            tc.tile_pool(name="wt", bufs=1) as wt_pool,
            tc.tile_pool(name="xbf", bufs=2 * NKG + 1) as xbf_pool,
            tc.tile_pool(name="psum", bufs=2, space="PSUM") as psum_pool,
            tc.tile_pool(name="osb", bufs=3) as osb_pool,
            tc.tile_pool(name="consts", bufs=1) as const_pool,
        ):
            # HAM warmup: dummy matmuls on zeroed SBUF while the first loads
            # are in flight, so the PE clock-gate (4/8 cold -> 8/8 warm after
            # ~3.4us of activity) opens before real matmuls start.
            zeros = const_pool.tile([128, NMAX], bf16, tag="zeros", name="zeros")
            nc.vector.memset(zeros[:], 0)
            psw = psum_pool.tile([128, NMAX], f32, tag="warm", name="warm", bufs=1)
            N_WARM = 22
            for i in range(N_WARM):
                nc.tensor.matmul(
                    psw[:, :],
                    zeros[:, 0:128],
                    zeros[:, :],
                    start=(i == 0),
                    stop=(i == N_WARM - 1),
                )

            # Startup: interleave weight dequant (int8 -> bf16 cast DMA, exact
            # for int8-range values) with the first s-chunk's x loads, x tile
            # first — the tensor engine needs (xg0, wtg0) for its first MM.
            # The very first (x, w) pair covers a single k-tile so the first
            # matmul's dependencies are a few hundred KB, not MBs.
            groups0 = [(0, 1), (1, kgrp - 1)] + [
                (g * kgrp, kgrp) for g in range(1, NKG)
            ]
            sc0 = chunk_sched[0]
            wtk = {}  # k -> (tile, idx within tile)
            xg0 = {}
            for gi, (k0, kn) in enumerate(groups0):
                t = xbf_pool.tile([128, kn, sc0], bf16, tag="xbf", name=f"x0_{gi}")
                src = xt[k0 * 128 : (k0 + kn) * 128, 0:sc0].rearrange(
                    "(g p) s -> p g s", p=128
                )
                nc.gpsimd.dma_start(out=t[:], in_=src)
                for i in range(kn):
                    xg0[k0 + i] = (t, i)
                wtile = wt_pool.tile(
                    [128, kn, out_per], bf16, tag=f"wt{gi}", name=f"wt{gi}"
                )
                wsrc = wt[k0 * 128 : (k0 + kn) * 128, :].rearrange(
                    "(g p) o -> p g o", p=128
                )
                nc.gpsimd.dma_start(out=wtile[:], in_=wsrc)
                for i in range(kn):
                    wtk[k0 + i] = (wtile, i)

            scale_sb = const_pool.tile([128, 1], f32, tag="scale", name="scale_sb")
            nc.sync.dma_start(out=scale_sb[:], in_=scale.partition_broadcast(128))
            bias_sb = const_pool.tile([128, out_per], f32, tag="bias", name="bias_sb")
            nc.sync.dma_start(out=bias_sb[:], in_=bias.partition_broadcast(128))

            s0 = 0
            for ci, sc in enumerate(chunk_sched):
                if ci == 0:
                    xg = xg0
                else:
                    # x chunk load: cast f32 -> bf16 in DMA, [128, kgrp, sc]
                    xg = {}
                    for g in range(NKG):
                        t = xbf_pool.tile(
                            [128, kgrp, sc], bf16, tag="xbf", name=f"x{ci}_{g}"
                        )
                        src = xt[
                            g * kgrp * 128 : (g + 1) * kgrp * 128, s0 : s0 + sc
                        ].rearrange("(g p) s -> p g s", p=128)
                        nc.gpsimd.dma_start(out=t[:], in_=src)
                        for i in range(kgrp):
                            xg[g * kgrp + i] = (t, i)

                for sub in range(sc // S_SUB):
                    psums = [
                        psum_pool.tile(
                            [128, NMAX], f32, tag=f"ps{j}", name=f"ps{ci}_{sub}_{j}"
                        )
                        for j in range(len(chunks))
                    ]
                    for k in range(KT):
                        xt_t, xi = xg[k]
                        w_t, wi = wtk[k]
                        lhsT = xt_t[:, xi, sub * 128 : (sub + 1) * 128]
                        for j, (off, sz) in enumerate(chunks):
                            nc.tensor.matmul(
                                psums[j][:, :sz],
                                lhsT,
                                w_t[:, wi, off : off + sz],
                                start=(k == 0),
                                stop=(k == KT - 1),
                            )
                    osb = osb_pool.tile(
                        [128, out_per], f32, tag="osb", name=f"o{ci}_{sub}"
                    )
                    r0 = s0 + sub * S_SUB
                    for j, (off, sz) in enumerate(chunks):
                        nc.vector.scalar_tensor_tensor(
                            osb[:, off : off + sz],
                            psums[j][:, :sz],
                            scale_sb[:, 0:1],
                            bias_sb[:, off : off + sz],
                            mybir.AluOpType.mult,
                            mybir.AluOpType.add,
                        )
                        nc.sync.dma_start(
                            out=out[r0 : r0 + S_SUB, off : off + sz],
                            in_=osb[:, off : off + sz],
                        )
                s0 += sc

    nc.compile()
    return nc


def _get_nc():
    key = "full"
    if key not in _cache:
        _cache[key] = build_nc()
    return _cache[key]


def kernel(x, weight_int8, scale, bias):
    global LAST_RESULT
    x = np.asarray(x, dtype=np.float32)
    w = np.asarray(weight_int8)
    scale_f = np.float32(np.asarray(scale).reshape(()))
    bias = np.asarray(bias, dtype=np.float32)

    # host-side layout prep (sharding): contraction dim to the front; the
    # int8-valued weight is shipped in its compressed (int8) form
    xt = np.ascontiguousarray(x.reshape(S_TOT, IN_F).T)  # [in, s]
    wt_full = np.ascontiguousarray(w.T.astype(np.int8))  # [in, out]
    scale_rep = np.full((1, 1), scale_f, dtype=np.float32)

    nc = _get_nc()
    in_maps = []
    for c in range(NCORES):
        o0, o1 = c * OUT_PER, (c + 1) * OUT_PER
        in_maps.append(
            {
                "xt": xt,
                "wt": np.ascontiguousarray(wt_full[:, o0:o1]),
                "bias": np.ascontiguousarray(bias[o0:o1][None, :]),
                "scale": scale_rep,
            }
        )

    res = run_bass_kernel_spmd(
        nc, in_maps, core_ids=list(range(NCORES)), trace=TRACE
    )
    LAST_RESULT = res
    out = np.concatenate([res.results[c]["out"] for c in range(NCORES)], axis=1)
    return out.reshape(B, S, OUT_F)
